# revision 26
# baseline (speedup 1.0000x reference)
"""Trainium2 Bass kernel for the DLI loss problem.

Math: in the reference, logits[b,j,k] = a[b,j] + t[b,j+2+k] + fc_b where
a = src_last @ Wh (the 2-step LSTM head) is constant over k. In
loss = LSE_k(logits) - logits[...,0] the a + fc_b terms cancel exactly, so

    loss[b,j] = log(sum_{m=j+2}^{T-1} exp(t[b,m])) - t[b,j+2]
    t[b,m]    = (seg_sum[b,m] @ We) / len[b,m],   We = fc_w[0, H:]

The kernel therefore streams encoder_output (the memory-bound part),
computes ragged segment sums via a PE matmul against an on-device-built
membership matrix, dots with We, and does the tiny LSE-tail epilogue.
Data-parallel over batch: 4 samples per core on 8 cores.

Device layout per sample: partition p holds rows s = 32p + n (n in 0..31),
so every DMA is contiguous per partition. Membership is built from
C[p,(n,u)] = sign(ends_ext[u] + 0.5 - n - 32p) and M = C[:, :, 1:] - C[:, :, :-1]
(values {0,2}; the 1/2 is folded into the host-provided recip = 0.5/len).
"""

import numpy as np
from contextlib import ExitStack

B, S, E, H, T = 32, 4096, 256, 256, 64
NCORES = 8
BPC = B // NCORES      # 4 samples per core
J = T - 2              # 62
NCH = 32               # chunks per sample; s = 32*p + n
U = T + 1              # 65 boundaries (ends_ext = [-1, ends...])
VLEN = NCH * U         # 2080

_cache = {}


def _build(reps=1):
    import concourse.bacc as bacc
    import concourse.tile as tile
    import concourse.mybir as mybir

    f32 = mybir.dt.float32
    bf16 = mybir.dt.bfloat16
    i32 = mybir.dt.int32
    Alu = mybir.AluOpType
    Act = mybir.ActivationFunctionType

    nc = bacc.Bacc("TRN2", target_bir_lowering=False, debug=False)
    enc = nc.dram_tensor("enc", [BPC, S, E], f32, kind="ExternalInput").ap()
    vth = nc.dram_tensor("vth", [BPC, 2, VLEN], f32, kind="ExternalInput").ap()
    lhs2 = nc.dram_tensor("lhs2", [128, 128], f32, kind="ExternalInput").ap()
    recip = nc.dram_tensor("recip", [T, BPC], f32, kind="ExternalInput").ap()
    weh_bc = nc.dram_tensor("weh_bc", [T, E], f32, kind="ExternalInput").ap()
    tri = nc.dram_tensor("tri", [128, J], f32, kind="ExternalInput").ap()
    lnsuf_o = nc.dram_tensor("lnsuf", [J, BPC], f32, kind="ExternalOutput").ap()
    tvals_o = nc.dram_tensor("tvals", [T, BPC], f32, kind="ExternalOutput").ap()

    with tile.TileContext(nc) as tc, ExitStack() as ctx:
        const = ctx.enter_context(tc.tile_pool(name="const", bufs=1))
        encp = ctx.enter_context(tc.tile_pool(name="encp", bufs=2))
        cp = ctx.enter_context(tc.tile_pool(name="cp", bufs=2))
        mp = ctx.enter_context(tc.tile_pool(name="mp", bufs=2))
        scrp = ctx.enter_context(tc.tile_pool(name="scr", bufs=2))
        smallp = ctx.enter_context(tc.tile_pool(name="small", bufs=2))
        segp = ctx.enter_context(tc.tile_pool(name="seg", bufs=2, space="PSUM"))
        vpsp = ctx.enter_context(tc.tile_pool(name="vps", bufs=1, space="PSUM"))
        psmall = ctx.enter_context(tc.tile_pool(name="psmall", bufs=1, space="PSUM"))

        # constants (matmuls padded to K=128 — K<128 crashes this runtime)
        lhs2_sb = const.tile([128, 128], f32)
        nc.sync.dma_start(lhs2_sb[:], lhs2[:])
        we_bc = const.tile([T, E], f32)
        nc.sync.dma_start(we_bc[:], weh_bc[:])
        tri_sb = const.tile([128, J], f32)
        nc.sync.dma_start(tri_sb[:], tri[:])
        recip_sb = const.tile([T, BPC], f32)
        nc.sync.dma_start(recip_sb[:], recip[:])
        t_all = const.tile([T, BPC], f32)
        e_all = const.tile([128, BPC], f32)
        nc.vector.memset(e_all[:], 0.0)

        # v rhs tile: rows 0..1 per-sample data, rows 2..127 stay zero
        vrhs = const.tile([128, VLEN], f32)
        nc.vector.memset(vrhs[:], 0.0)

        for b in [b for _ in range(reps) for b in range(BPC)]:
            # SWDGE DMA casts f32 -> bf16 in flight (HBM read traffic unchanged)
            enc_t = encp.tile([128, NCH, E], bf16)
            src = enc[b].rearrange("(p n) e -> p n e", p=128)
            for h in range(2):
                nc.gpsimd.dma_start(enc_t[:, h * 16:(h + 1) * 16, :],
                                    src[:, h * 16:(h + 1) * 16, :])

            nc.sync.dma_start(vrhs[0:2, :], vth[b])
            # v_ps[p, c] = v[c]*1 + 32p*(-1) via K=128 fp32 matmul (rows 2+ zero)
            v_ps = vpsp.tile([128, VLEN], f32)
            for k in range(5):
                sl = slice(k * 512, min((k + 1) * 512, VLEN))
                nc.tensor.matmul(v_ps[:, sl], lhsT=lhs2_sb[:], rhs=vrhs[:, sl],
                                 start=True, stop=True)

            # C[p,(n,u)] = sign(v[(n,u)] - 32p) in {-1, +1}
            C = cp.tile([128, NCH, U], bf16)
            nc.scalar.sign(C[:], v_ps[:].rearrange("p (n u) -> p n u", u=U))
            # M[p,(n,t)] = C[p,(n,t+1)] - C[p,(n,t)] in {0, 2}
            M = mp.tile([128, NCH, T], bf16)
            nc.vector.tensor_tensor(M[:], C[:, :, 1:U], C[:, :, 0:T], Alu.subtract)

            # 2*seg_sum[t, e] accumulated over the 32 chunks
            seg_ps = segp.tile([T, E], f32)
            for n in range(NCH):
                nc.tensor.matmul(
                    seg_ps[:], lhsT=M[:, n, :], rhs=enc_t[:, n, :],
                    start=(n == 0), stop=(n == NCH - 1),
                )

            # t_sum[t] = sum_e seg[t,e] * We[e]; t = t_sum * (0.5/len)
            scr = scrp.tile([T, E], f32)
            nc.vector.tensor_tensor(scr[:], seg_ps[:], we_bc[:], Alu.mult)
            tsum = smallp.tile([T, 1], f32, tag="tsum")
            nc.vector.tensor_reduce(tsum[:], scr[:], axis=mybir.AxisListType.X,
                                    op=Alu.add)
            nc.vector.tensor_tensor(t_all[:, b:b + 1], tsum[:],
                                    recip_sb[:, b:b + 1], Alu.mult)
            nc.scalar.activation(e_all[0:T, b:b + 1], t_all[:, b:b + 1], Act.Exp)

        # suffix sums over exp(t): suf[j,b] = sum_{m>=j+2} e[m,b]
        suf_ps = psmall.tile([J, BPC], f32, tag="ps_small")
        nc.tensor.matmul(suf_ps[:], lhsT=tri_sb[:], rhs=e_all[:],
                         start=True, stop=True)
        lnsuf_sb = const.tile([J, BPC], f32)
        nc.scalar.activation(lnsuf_sb[:], suf_ps[:], Act.Ln)
        nc.sync.dma_start(lnsuf_o[:], lnsuf_sb[:])
        nc.sync.dma_start(tvals_o[:], t_all[:])

    nc.compile()
    return nc


def _get_nc(reps=1):
    key = ("nc", reps)
    if key not in _cache:
        _cache[key] = _build(reps)
    return _cache[key]


def _host_prep(ends_all):
    """Per-sample threshold rows, recips, and the triangular constant."""
    n_idx = np.arange(NCH, dtype=np.float64)
    vths = np.empty((B, 2, VLEN), np.float32)
    recips = np.empty((B, T), np.float32)
    for b in range(B):
        ends = ends_all[b].astype(np.float64)
        ends_ext = np.concatenate([[-1.0], ends])            # (65,)
        v = ends_ext[None, :] + 0.5 - n_idx[:, None]         # (32, 65)
        vths[b, 0] = v.reshape(-1).astype(np.float32)
        vths[b, 1] = -1.0
        lens = ends - ends_ext[:T]
        recips[b] = (0.5 / lens).astype(np.float32)
    tri = np.zeros((128, J), np.float32)
    tri[:T] = (np.arange(T)[:, None] >= np.arange(J)[None, :] + 2).astype(np.float32)
    lhs2 = np.zeros((128, 128), np.float32)
    lhs2[0] = 1.0
    lhs2[1] = 32.0 * np.arange(128)
    return vths, recips, tri, lhs2


def kernel(**inputs):
    from concourse.bass_utils import run_bass_kernel_spmd

    enc = np.ascontiguousarray(inputs["encoder_output"], dtype=np.float32)
    ends_all = np.asarray(inputs["his_turn_end_ids"]).astype(np.int64)
    We = np.ascontiguousarray(inputs["fc_w"][0, H:], dtype=np.float32)

    vths, recips, tri, lhs2 = _host_prep(ends_all)

    in_maps = []
    for c in range(NCORES):
        sl = slice(c * BPC, (c + 1) * BPC)
        in_maps.append({
            "enc": enc[sl],
            "vth": np.ascontiguousarray(vths[sl]),
            "recip": np.ascontiguousarray(recips[sl].T),
            "weh_bc": np.ascontiguousarray(np.broadcast_to(We[None, :], (T, E))),
            "tri": tri,
            "lhs2": lhs2,
        })

    nc = _get_nc()
    res = run_bass_kernel_spmd(nc, in_maps, list(range(NCORES)))

    total = 0.0
    for c in range(NCORES):
        lnsuf = res.results[c]["lnsuf"].astype(np.float64)   # (J, BPC)
        tvals = res.results[c]["tvals"].astype(np.float64)   # (T, BPC)
        total += (lnsuf - tvals[2:, :]).sum()
    return np.float32(total / (B * J))


if __name__ == "__main__":
    data = dict(np.load("/root/problem/_inputs.npz"))
    out = kernel(**data)
    print("kernel out:", out)
